# revision 15
# baseline (speedup 1.0000x reference)
"""CaptioningRNN forward loss on 8 Trainium2 NeuronCores.

Math (per reference):
    h0 = features @ W_proj + b_proj                       (no tanh)
    x  = W_embed[captions[:, :-1]]
    a  = x @ Wx + b                                       (precomputed input drive)
    h_t = tanh(h_{t-1} @ Wh + a_t)                        (T sequential steps)
    s  = h @ W_out + b_out                                (N*T x V logits)
    loss = sum over (n,t) of mask * (logsumexp(s) - s[target]) / N

Sharding: data-parallel over batch N=256 -> 32 rows/core, weights replicated.
Each core returns sumexp[r] = sum_v exp(s_rv) and st[r] = s_r,target; the
host finishes with sum(mask * (log(sumexp) - st)) / N (cheap glue: 16K logs).

On-chip (per core, t-major rows r = t*32 + n, tiled [128, m] with r=m*128+i):
  * xT produced directly by batched dma_gather(transpose=True) from W_embed
    (token indices in the 16-partition-wrapped, 8x-replicated int16
    layout); no PE transposes needed.  The HW transpose-gather ucode
    breaks above 512 indices, so 4 gathers of 512 + one 128-token
    pre-gather that lets the recurrence start before the bulk loads land.
  * recurrence in transposed form: one [128,128] PSUM tile per step
    (4 h'-chunks side by side); the input drive Wx @ x_t is contracted
    directly in the same accumulation group as Wh @ h_{t-1} (no aT
    precompute, no extra PSUM pool).
  * scores: per 128-row tile, 7 vocab groups (6x1536 + 784); fp8 DoubleRow
    matmuls into PSUM, ACT Exp into a bf16 accumulator tile; groups 1..6
    summed in on DVE (2x-mode tensor_tensor adds) and one 1x tensor_reduce
    per row tile produces sumexp.  No accum_out -> no 187ns ACT reads
    (except the last tile, where accum_out shortens the tail chain).
  * scores run as a flat (tile, group) work list consumed ~1.5 windows
    behind the recurrence, so no exp ever waits on a tanh through the
    in-order PE queue; W_out streams in 10 group-order 1024-wide chunks
    whose DMA priority is pushed past the ramp-critical loads.
  * target score: rows of W_out.T fetched by one batched dma_gather per 4
    row tiles; dot with h rows (DMA-transposed) on DVE, lagged 4 windows
    so the late wg gather can't head-of-line-block the DVE queue.
"""

import sys

for _p in ("/opt/trn_rl_repo", "/root/.axon_site/_ro/trn_rl_repo"):
    if _p not in sys.path:
        sys.path.insert(0, _p)

import numpy as np
import ml_dtypes
from contextlib import ExitStack

import concourse.bass as bass
import concourse.tile as tile
from concourse import bacc, mybir
from concourse.bass_utils import run_bass_kernel_spmd

F32 = mybir.dt.float32
BF16 = mybir.dt.bfloat16
FP8 = mybir.dt.float8e4
I16 = mybir.dt.int16
AF = mybir.ActivationFunctionType
OP = mybir.AluOpType

# Problem sizes (hardcoded per spec).
N, T, D, W, H, V = 256, 64, 1280, 256, 512, 10000
NCORES = 8
NS = N // NCORES          # 32 batch rows per core
R = NS * T                # 2048 (t-major rows per core)
MT = R // 128             # 16 row tiles
KH = H // 128             # 4 hidden chunks
KW = W // 128             # 2 embed chunks
KD = D // 128             # 10 feature chunks
TSLOT = T + 1             # h slots (0 = h0)
HTB = TSLOT * NS          # 2080 columns per hidden-chunk block of hT
WGW_BO = 640              # gathered W_out^T row incl b_out col (256B mult)
WGW = 512                 # gathered W_out^T row, zero b_out (common case)
P = 128
NXG = 4                   # x-token gathers (512 idxs each; the HW
                          # transpose-gather ucode breaks above 512)
NWG = 4                   # target-token gathers (512 idxs each)

# vocab tiling: 512-wide matmuls, grouped into <=1536-wide exp groups
_VT = []
v = 0
while v < V:
    w = min(512, V - v)
    _VT.append((v, w))
    v += w
VGROUPS = []          # list of list[(voff, width)]
i = 0
while i < len(_VT):
    VGROUPS.append(_VT[i:i + 3])
    i += 3
NG = len(VGROUPS)     # 7 (6x1536 + 784)

_CACHE = {}
_WOUT_SPLIT = 2500  # DMA chunk width for W_out load


def _build(with_bout_mm: bool, zero_bias: bool = True):
    """Build + compile the per-core Bass program (identical across cores)."""
    nc = bacc.Bacc(
        "TRN2", target_bir_lowering=False, debug=False, num_devices=NCORES
    )

    featT = nc.dram_tensor("featT", [D, NS], FP8, kind="ExternalInput")
    tok16 = nc.dram_tensor("tok16", [P, 256], I16, kind="ExternalInput")
    w_out = nc.dram_tensor("w_out", [H, V], FP8, kind="ExternalInput")
    wgw = WGW_BO if with_bout_mm else WGW
    w_outT = nc.dram_tensor("w_outT", [V, wgw], BF16, kind="ExternalInput")
    w_embed = nc.dram_tensor("w_embed", [V, W], BF16, kind="ExternalInput")
    wh_d = nc.dram_tensor("wh", [H, H], BF16, kind="ExternalInput")
    wx_d = nc.dram_tensor("wx", [W, H], BF16, kind="ExternalInput")
    wproj_d = nc.dram_tensor("wproj", [D, H], FP8, kind="ExternalInput")
    b_d = nc.dram_tensor("b", [H, 1], F32, kind="ExternalInput")
    bp_d = nc.dram_tensor("b_proj", [H, 1], F32, kind="ExternalInput")
    bo_d = nc.dram_tensor("b_out_row", [1, V], F32, kind="ExternalInput")
    sumexp_d = nc.dram_tensor("sumexp", [P, MT], F32, kind="ExternalOutput")
    st_d = nc.dram_tensor("st", [P, MT], F32, kind="ExternalOutput")

    with tile.TileContext(nc) as tc, ExitStack() as ctx:
        const = ctx.enter_context(tc.tile_pool(name="const", bufs=1))
        work = ctx.enter_context(tc.tile_pool(name="work", bufs=3))
        psum_sc = ctx.enter_context(tc.tile_pool(name="psc", bufs=2, space="PSUM"))
        psum_st = ctx.enter_context(tc.tile_pool(name="pst", bufs=1, space="PSUM"))

        # ---- persistent SBUF tensors ----
        wout_sb = const.tile([P, KH * V], FP8)       # 40KB/part
        hT8 = const.tile([P, KH * R], FP8)           # fp8 copy of hT slots 1..64
        hT = const.tile([P, KH * HTB], BF16)         # 16.6KB/part
        wg_sb = const.tile([P, MT * wgw], BF16)      # gathered target W_out rows
        h_rows = const.tile([P, MT * H], BF16)       # h row-major (DMA-transposed)
        # xT in two tiles (one per gather: transposed gather output must be
        # a fully contiguous [128, KW, NI] block)
        xT_sb = [const.tile([P, KW * (R // NXG)], BF16, name=f"xT{g}")
                 for g in range(NXG)]
        wh_sb = const.tile([P, KH * KH * P], BF16)
        wx_sb = const.tile([P, KW * KH * P], BF16)
        wproj_sb = const.tile([P, KD * KH * P], FP8)
        featT_sb = const.tile([P, KD * NS], FP8)
        tok16_sb = const.tile([P, 256], I16)
        b_sb = const.tile([P, KH], F32)
        bp_sb = const.tile([P, KH], F32)
        sumexp = const.tile([P, MT], F32)
        st_all = const.tile([P, MT], F32)
        warm = const.tile([P, 1], F32)
        if with_bout_mm:
            bo_sb = const.tile([1, V], F32)
            onesr = const.tile([1, P], F32)

        # ---- DMAs: token indices first (they gate the gathers -> aT ->
        # recurrence chain), then small weights, then W_out (10MB) on the
        # ScalarE HWDGE queue.  Each weight is one DMA with a 3D access
        # pattern (k-chunk dim folded in) to cut HWDGE fixed overheads.
        # DMA priority: everything that gates the recurrence goes first on
        # the SP HWDGE queue (tok16 -> gathers; wh/wx -> steps; featT/wproj
        # (fp8) -> h0).  The 5.1MB W_out load rides the pool SWDGE queue
        # BEHIND the x gathers, so its 4x3.2us transfers can't starve the
        # ramp-critical path on the shared DMA engines; wg gathers go last
        # (their consumer, the target-score dot, also runs on Pool and is
        # only needed by the final output DMA).
        nc.sync.dma_start(tok16_sb[:], tok16[:, :])
        nc.sync.dma_start(featT_sb[:],
                          bass.AP(featT, 0, [[NS, P], [P * NS, KD], [1, NS]]))
        nc.sync.dma_start(wproj_sb[:],
                          bass.AP(wproj_d, 0, [[H, P], [P * H, KD], [1, H]]))
        nc.sync.dma_start(wx_sb[:], bass.AP(wx_d, 0, [[H, P], [P * H, KW], [1, H]]))
        nc.sync.dma_start(wh_sb[:], bass.AP(wh_d, 0, [[H, P], [P * H, KH], [1, H]]))
        if not zero_bias:
            nc.sync.dma_start(b_sb[:], bass.AP(b_d, 0, [[1, P], [P, KH]]))
            nc.sync.dma_start(bp_sb[:], bass.AP(bp_d, 0, [[1, P], [P, KH]]))
        if with_bout_mm:
            nc.scalar.dma_start(bo_sb[:], bo_d[:, :])
            nc.gpsimd.memset(onesr[:], 1.0)

        # ---- batched gathers (SWDGE): xT directly in transposed layout.
        # A 128-token pre-gather covers steps 1..4 so the recurrence can
        # start ~3us before the bulk gathers + W_out land. ----
        NI = R // NXG
        xTp = const.tile([P, KW * P], BF16)
        xTp3 = xTp[:].rearrange("p (k r) -> p k r", k=KW)
        nc.gpsimd.dma_gather(
            out_ap=xTp3[:, :, :], in_ap=w_embed[:, :],
            idxs_ap=tok16_sb[:, 0:8],
            num_idxs=P, num_idxs_reg=P, elem_size=W, transpose=True)
        xT3 = [t[:].rearrange("p (k r) -> p k r", k=KW) for t in xT_sb]
        for g in range(NXG):
            nc.gpsimd.dma_gather(
                out_ap=xT3[g][:, :, :],
                in_ap=w_embed[:, :],
                idxs_ap=tok16_sb[:, g * (NI // 16):(g + 1) * (NI // 16)],
                num_idxs=NI, num_idxs_reg=NI, elem_size=W, transpose=True)
        # W_out in fine-grained 1024-wide chunks: the shared DMA engines
        # serve requests FIFO, so small chunks interleave with (rather than
        # block) the ramp-critical gather/weight transfers, and chunk k
        # still lands roughly when the first exps need it (consumption
        # order matches vocab order)
        wout3 = wout_sb[:].rearrange("p (k c) -> p k c", k=KH)
        for ci, v0 in enumerate(range(0, V, 1024)):
            wd = min(1024, V - v0)
            inst = nc.sync.dma_start(
                wout3[:, :, v0:v0 + wd],
                bass.AP(w_out, v0, [[V, P], [P * V, KH], [1, wd]]))
            # explicitly AFTER the ramp-critical loads/gathers but well
            # before the exps that consume chunk ci (the scheduler would
            # otherwise greedily hoist these 1.5us transfers into the
            # entry-critical DMA window)
            inst.bass_priority = 150 + 25 * ci
        wg3 = wg_sb[:].rearrange("p (m e) -> p m e", e=wgw)
        NJ = R // NWG
        MB = MT // NWG
        for g in range(NWG):
            inst = nc.gpsimd.dma_gather(
                out_ap=wg3[:, g * MB:(g + 1) * MB, :],
                in_ap=w_outT[:, :],
                idxs_ap=tok16_sb[:, 128 + g * (NJ // 16):128 + (g + 1) * (NJ // 16)],
                num_idxs=NJ, num_idxs_reg=NJ, elem_size=wgw)
            inst.bass_priority = 800 + 10 * g

        # warm the ACT exp/tanh table set early (overlaps the big DMAs)
        nc.gpsimd.memset(warm[:], 0.0)
        nc.scalar.activation(warm[:], warm[:], AF.Exp)

        ident = const.tile([P, P], BF16)
        from concourse.masks import make_identity
        make_identity(nc, ident[:])
        # warm the PE out of its low p-state before the first real matmuls
        # (the cost model runs cold matmuls ~2-4x slower; a ~2us chain of
        # junk transposes brings the pipeline to full speed by ~4us)
        for _w in range(14):
            pw = psum_st.tile([P, P], BF16, tag="step")
            nc.tensor.transpose(pw[:], ident[:], ident[:])

        hT3 = hT[:].rearrange("p (b c) -> p b c", b=KH)
        hT8_3 = hT8[:].rearrange("p (b c) -> p b c", b=KH)

        # ---- h0 first: PE's earliest work, gated only on wproj/featT ----
        for mp in range(KH):
            ps = psum_st.tile([P, NS], F32, tag="step")
            for k in range(KD):
                nc.tensor.matmul(ps[:], lhsT=wproj_sb[:, (k * KH + mp) * P:
                                                      (k * KH + mp + 1) * P],
                                 rhs=featT_sb[:, k * NS:(k + 1) * NS],
                                 start=(k == 0), stop=(k == KD - 1))
            if zero_bias:
                nc.vector.tensor_copy(hT[:, mp * HTB:mp * HTB + NS], ps[:])
            else:
                nc.scalar.activation(hT[:, mp * HTB:mp * HTB + NS], ps[:],
                                     AF.Identity, bias=bp_sb[:, mp:mp + 1])

        def emit_step(t):
            """h slot t (1..T) from slot t-1; PSUM [128, 4*NS], blocks =
            h'-chunks.  The input drive Wx @ x_t is contracted directly in
            the same accumulation group as Wh @ h_{t-1} (no precomputed aT
            tile, no PSUM-pool contention for it).  Steps 1..4 read x from
            the 128-token pre-gather so the recurrence starts before the
            bulk gathers land."""
            ps = psum_st.tile([P, KH * NS], F32, tag="step")
            c0 = (t - 1) * NS
            for mp in range(KH):
                for k in range(KW):
                    x3 = xTp3 if t <= 4 else xT3[c0 // NI]
                    nc.tensor.matmul(
                        ps[:, mp * NS:(mp + 1) * NS],
                        lhsT=wx_sb[:, (k * KH + mp) * P:(k * KH + mp + 1) * P],
                        rhs=x3[:, k, c0 % NI if t > 4 else c0:
                               (c0 % NI if t > 4 else c0) + NS],
                        start=(k == 0), stop=False, skip_group_check=True)
                for k in range(KH):
                    nc.tensor.matmul(
                        ps[:, mp * NS:(mp + 1) * NS],
                        lhsT=wh_sb[:, (k * KH + mp) * P:(k * KH + mp + 1) * P],
                        rhs=hT[:, k * HTB + (t - 1) * NS:k * HTB + t * NS],
                        start=False, stop=(mp == KH - 1 and k == KH - 1),
                        skip_group_check=True)
            ps3 = ps[:].rearrange("p (b n) -> p b n", b=KH)
            if zero_bias:
                nc.scalar.activation(hT3[:, :, t * NS:(t + 1) * NS], ps3[:],
                                     AF.Tanh)
            else:
                for mp in range(KH):
                    nc.scalar.activation(
                        hT3[:, mp, t * NS:(t + 1) * NS],
                        ps3[:, mp, :], AF.Tanh, bias=b_sb[:, mp:mp + 1])
            # fp8 shadow of h_t for the DoubleRow score matmuls
            nc.vector.tensor_copy(hT8_3[:, :, (t - 1) * NS:t * NS],
                                  hT3[:, :, t * NS:(t + 1) * NS])

        # Per-row-tile exp accumulator: exp of group 0 writes it directly,
        # later groups exp into a scratch tile and are summed in on DVE
        # (bf16 tensor_tensor adds run in the DVE 2x perf mode).  One final
        # 1x tensor_reduce per tile produces sum_v exp(s).  This keeps ACT
        # free of per-instruction accum_out reads.
        acc_tiles = {}
        strip = const.tile([P, NG], F32)   # last-tile accum_out partials

        def emit_scores(m, gi):
            ps = psum_sc.tile([P, 1536], F32, tag="sc")
            off = 0
            for (voff, wd) in VGROUPS[gi]:
                for g in range(KH // 2):
                    nc.tensor.matmul(
                        ps[:, off:off + wd],
                        lhsT=hT8_3[:, 2 * g:2 * g + 2, m * P:(m + 1) * P],
                        rhs=wout3[:, 2 * g:2 * g + 2, voff:voff + wd],
                        start=(g == 0),
                        stop=(g == KH // 2 - 1 and not with_bout_mm),
                        perf_mode=mybir.MatmulPerfMode.DoubleRow)
                if with_bout_mm:
                    nc.tensor.matmul(
                        ps[:, off:off + wd],
                        lhsT=onesr[:, :],
                        rhs=bo_sb[:, voff:voff + wd],
                        start=False, stop=True,
                        skip_group_check=True)
                off += wd
            if m == MT - 1:
                # last tile: ACT accum_out (187ns aux reads) instead of DVE
                # adds -- the tail chain shrinks to one tiny 7-wide reduce
                esc = work.tile([P, 1536], BF16, tag="esc")
                nc.scalar.activation(esc[:, :off], ps[:, :off], AF.Exp,
                                     accum_out=strip[:, gi:gi + 1])
            elif gi == 0:
                acc = work.tile([P, 1536], BF16, tag="acc")
                acc_tiles[m] = acc
                nc.scalar.activation(acc[:, :off], ps[:, :off], AF.Exp)
            else:
                esc = work.tile([P, 1536], BF16, tag="esc")
                nc.scalar.activation(esc[:, :off], ps[:, :off], AF.Exp)
                acc = acc_tiles[m]
                nc.vector.tensor_add(acc[:, :off], acc[:, :off], esc[:, :off])

        def emit_reduce(m):
            if m == MT - 1:
                nc.vector.tensor_reduce(sumexp[:, m:m + 1], strip[:],
                                        axis=mybir.AxisListType.X, op=OP.add)
                return
            acc = acc_tiles.pop(m)
            nc.vector.tensor_reduce(sumexp[:, m:m + 1], acc[:],
                                    axis=mybir.AxisListType.X, op=OP.add)

        def emit_hrows(m):
            for k in range(KH):
                nc.sync.dma_start_transpose(
                    h_rows[:, m * H + k * P:m * H + (k + 1) * P],
                    hT[:, k * HTB + NS + m * P:k * HTB + NS + (m + 1) * P])

        def emit_st(m):
            """Target-score dot for tile m.  Runs 4 windows behind the
            h_rows transpose so the wg gather (last in the DMA priority
            order) can never head-of-line-block the DVE queue."""
            junk = work.tile([P, H], BF16, tag="junk")
            nc.vector.tensor_mul(junk[:], h_rows[:, m * H:(m + 1) * H],
                                 wg_sb[:, m * wgw:m * wgw + H])
            if with_bout_mm:
                stp = work.tile([P, 1], F32, tag="stp")
                nc.vector.tensor_reduce(stp[:], junk[:],
                                        axis=mybir.AxisListType.X, op=OP.add)
                # + b_out[target] from the augmented gather column
                nc.vector.tensor_add(st_all[:, m:m + 1], stp[:],
                                     wg_sb[:, m * wgw + H:m * wgw + H + 1])
            else:
                nc.vector.tensor_reduce(st_all[:, m:m + 1], junk[:],
                                        axis=mybir.AxisListType.X, op=OP.add)

        # ---- main loop.  Window m runs steps 4m+1..4m+4 and the scores of
        # row tile m-2: the two-window lag means every score matmul's hT8
        # inputs were finished a full window earlier, so the in-order PE
        # never stalls the exp stream on a tanh.  aT chunk c lands in
        # window c (chunk 0 pre-split before the loop so step 1 starts
        # early). ----
        # Flat (tile, group) scores work list consumed by a cursor: window
        # m may emit items of tiles <= m-1 (their steps finished a window
        # earlier).  Window 1 takes 4 items (fills the otherwise-idle ACT
        # during the early recurrence, rate-matched to the W_out chunk
        # arrivals); later windows take NG items, so the scores stream runs
        # ~1.5 windows behind the recurrence and the trailing tail after
        # the last step is only ~10 items.
        witems = [(t, g) for t in range(MT) for g in range(NG)]
        cursor = 0

        def emit_items(n):
            nonlocal cursor
            for (t, g) in witems[cursor:cursor + n]:
                emit_scores(t, g)
                if g == NG - 1:
                    emit_reduce(t)
            cursor += n

        for m in range(MT):
            want = 0 if m <= 1 else NG
            take = max(0, min(want, m * NG - cursor))
            q, r = divmod(take, 4)
            split = [q + (1 if j < r else 0) for j in range(4)]
            for j in range(4):
                emit_step(4 * m + j + 1)
                emit_items(split[j])
            emit_hrows(m)
            if m >= 4:
                emit_st(m - 4)
        emit_items(len(witems) - cursor)
        for mm_ in range(MT - 4, MT):
            emit_st(mm_)

        # ---- ship per-row sumexp and target scores; host does
        # mask * (log(sumexp) - st) ----
        nc.sync.dma_start(sumexp_d[:, :], sumexp[:])
        nc.sync.dma_start(st_d[:, :], st_all[:])

    nc.compile()
    return nc


def _wrap16(flat: np.ndarray, nblk: int) -> np.ndarray:
    """Wrap a flat int index list into the dma_gather layout: token f of
    block g at [f%16, g*cols + f//16], with the 16-partition block
    replicated down all 128 partitions (each DMA-engine group reads its
    own copy -- unreplicated rows silently gather row 0 on HW)."""
    n = flat.shape[0] // nblk
    cols = n // 16
    out = np.zeros((16, nblk * cols), dtype=np.int16)
    for g in range(nblk):
        out[:, g * cols:(g + 1) * cols] = flat[g * n:(g + 1) * n].reshape(cols, 16).T
    return np.tile(out, (8, 1))


def _prepare_inputs(inputs):
    """Cast/shard host-side. Returns per-core in_maps plus host-side mask."""
    feats = np.asarray(inputs["features"], dtype=np.float32)
    cap = np.asarray(inputs["captions"])
    W_proj = np.asarray(inputs["W_proj"], dtype=np.float32)
    b_proj = np.asarray(inputs["b_proj"], dtype=np.float32).reshape(H, 1)
    W_embed = np.asarray(inputs["W_embed"], dtype=np.float32)
    Wx = np.asarray(inputs["Wx"], dtype=np.float32)
    Wh = np.asarray(inputs["Wh"], dtype=np.float32)
    b = np.asarray(inputs["b"], dtype=np.float32).reshape(H, 1)
    W_out = np.asarray(inputs["W_out"], dtype=np.float32)
    b_out = np.asarray(inputs["b_out"], dtype=np.float32)

    bf = ml_dtypes.bfloat16
    f8 = ml_dtypes.float8_e4m3
    w_out_f8 = np.ascontiguousarray(W_out.astype(f8))
    w_embed_bf = np.ascontiguousarray(W_embed.astype(bf))
    wh_bf = np.ascontiguousarray(Wh.astype(bf))
    wx_bf = np.ascontiguousarray(Wx.astype(bf))
    wproj_f8 = np.ascontiguousarray(W_proj.astype(f8))
    wgw = WGW_BO if np.any(b_out != 0.0) else WGW
    w_outT = np.zeros((V, wgw), dtype=bf)
    w_outT[:, :H] = W_out.T.astype(bf)
    if wgw == WGW_BO:
        w_outT[:, H] = b_out.astype(bf)
    bo_row = np.ascontiguousarray(b_out.reshape(1, V))

    shared = {
        "w_out": w_out_f8, "w_outT": w_outT, "w_embed": w_embed_bf,
        "wh": wh_bf, "wx": wx_bf, "wproj": wproj_f8,
        "b": b, "b_proj": b_proj, "b_out_row": bo_row,
    }
    in_maps = []
    masks = []
    for c in range(NCORES):
        rows = slice(c * NS, (c + 1) * NS)
        featT_c = np.ascontiguousarray(feats[rows].T.astype(f8))
        cin = np.asarray(cap[rows, :T], dtype=np.int32)     # [NS, T]
        cout = np.asarray(cap[rows, 1:T + 1], dtype=np.int32)
        # t-major flat r = t*NS + n
        fin = cin.T.reshape(R)
        fout = cout.T.reshape(R)
        tok16 = np.zeros((P, 256), dtype=np.int16)
        tok16[:, :128] = _wrap16(fin, NXG)
        tok16[:, 128:] = _wrap16(fout, NWG)
        # [128, MT] mask in the same layout the kernel writes sumexp/st
        tout = np.ascontiguousarray(fout.reshape(MT, P).T)
        masks.append((tout != 0).astype(np.float64))
        in_maps.append({**shared, "featT": featT_c, "tok16": tok16})
    zero_bias = not (np.any(b) or np.any(b_proj))
    return in_maps, masks, (bool(np.any(b_out != 0.0)), zero_bias)


def _get_program(flags=(False, True)):
    key = ("nc",) + tuple(flags)
    if key not in _CACHE:
        _CACHE[key] = _build(*flags)
    return _CACHE[key]


def kernel(**inputs) -> np.ndarray:
    in_maps, masks, flags = _prepare_inputs(inputs)
    nc = _get_program(flags)
    out = run_bass_kernel_spmd(nc, in_maps, core_ids=list(range(NCORES)))
    total = 0.0
    for c, r in enumerate(out.results):
        se = np.asarray(r["sumexp"], dtype=np.float64)
        st = np.asarray(r["st"], dtype=np.float64)
        total += float((masks[c] * (np.log(se) - st)).sum())
    return np.float32(total / N)
